# revision 1
# baseline (speedup 1.0000x reference)
"""Trainium2 Bass kernel for nn_ContrastiveLossWithAttention.

Contract: kernel(**inputs) takes the FULL unsharded inputs (as produced by
reference.setup_inputs) and returns the FULL output (a float32 scalar).

Sharding: pure data parallel — batch dim B=16 split as 2 batches per core
across 8 NeuronCores. Each core reduces its two 2048x2048 pred slabs to two
per-row/per-col vectors (T1row, T1col); the host applies the O(B*N) scalar
epilogue and the final scalar reduction across cores.

Algorithm (validated vs the reference to ~1e-7 in fp64/fp32 numpy):
  gt_perm is a permutation ground truth: one 1 per valid row (identity
  restricted to rows i < src_ns here; verified exactly host-side, with a
  numpy fallback if the structure doesn't hold). Under that structure the
  loss collapses to threshold sums over pred alone:
    row_gt[i] = clip(p[i,i]) for i < s          (diagonal)
    src_pos   = row_gt^2
    T1row[i]  = sum_j 1{pred_c >= row_gt[i]-beta} * s2m[i,j]
    src_neg   = T1row - src_pos
    T1col[j]  = sum_i 1{pred_c >= col_gt[j]-beta} * s2m[i,j]   (col_gt == row_gt vec)
    corr      = sum_{j<s} (T1col[j] - col_gt[j]^2)
    loss_b    = -0.5 * sum_{i<s} [ln(src_pos_i) - ln(1 + src_neg_i + corr)]
  with pred_c = clip(pred,0,1)*1{j<t}, s2m = (pred_c * 1{i<s})^2.

Host prep is O(B*N) vector math + one clip/cast pass (sharding/padding):
  p16   = bf16(clip(pred,0,1)) with the ragged column tail [tgt_ns:] zeroed
  thr_r = f32 row thresholds  clip(diag)*rowmask - beta   (STT scalar operand)
  thrc  = bf16 of the same vector (column thresholds, broadcast on device)
  rmask = f32 row-validity mask (Square scale operand)
Device does all O(N^2) work per 128-row chunk: the row-threshold sum (DVE
scalar_tensor_tensor with per-partition accumulate), the col indicator and
product (DVE tensor_tensor, bf16 2x mode), Square on ACT, and PE ones^T@
column sums accumulated in PSUM across the 16 chunks.
"""

import numpy as np
import ml_dtypes

B, N, M = 16, 2048, 2048
NCORES = 8
BPC = B // NCORES      # batches per core
PT = 128               # partitions
CH = N // PT           # row chunks per batch
CHR = 12               # row chunks actually computed: src_ns < 1537 always
                       # (setup_inputs range), so rows >= 1536 are never valid;
                       # guarded in kernel() with a fallback if violated
NR = PT * CHR          # rows computed on device
NQ = 4                 # 512-wide column slices for PE column sums

_cache = {}


def _build_program():
    import concourse.tile as tile
    from concourse import bacc, mybir

    f32 = mybir.dt.float32
    bf16 = mybir.dt.bfloat16
    Alu = mybir.AluOpType
    Act = mybir.ActivationFunctionType

    nc = bacc.Bacc("TRN2", debug=False, num_devices=NCORES)

    p_d = nc.dram_tensor("p16", [BPC, N, M], bf16, kind="ExternalInput")
    thr_d = nc.dram_tensor("thr_r", [BPC, NR], f32, kind="ExternalInput")
    thc_d = nc.dram_tensor("thrc16", [BPC, M], bf16, kind="ExternalInput")
    t1r_d = nc.dram_tensor("t1row", [BPC, NR], f32, kind="ExternalOutput")
    t1c_d = nc.dram_tensor("t1col", [BPC, M], f32, kind="ExternalOutput")

    with tile.TileContext(nc) as tc:
        with (
            tc.tile_pool(name="consts", bufs=1) as consts,
            tc.tile_pool(name="pb", bufs=2) as pb,
            tc.tile_pool(name="io", bufs=4) as io,
            tc.tile_pool(name="work", bufs=3) as work,
            tc.tile_pool(name="ps_col", bufs=2, space="PSUM") as ps_col,
        ):
            ones16 = consts.tile([PT, 1], bf16, tag="ones16")
            nc.vector.memset(ones16, 1.0)

            for b in range(BPC):
                thr_r = pb.tile([PT, CHR], f32, tag="thr_r")
                nc.sync.dma_start(out=thr_r, in_=thr_d[b].rearrange("(k p) -> p k", p=PT))
                thrc = pb.tile([PT, M], bf16, tag="thrc")
                nc.sync.dma_start(
                    out=thrc, in_=thc_d[b:b + 1, :].to_broadcast([PT, M])
                )

                t1c_ps = ps_col.tile([1, M], f32, tag="t1col")
                t1row = pb.tile([PT, CHR], f32, tag="t1row")
                for k in range(CHR):
                    predc = io.tile([PT, M], bf16, tag="predc")
                    nc.sync.dma_start(out=predc, in_=p_d[b, k * PT:(k + 1) * PT, :])
                    s2m = work.tile([PT, M], bf16, tag="s2m")
                    nc.scalar.activation(out=s2m, in_=predc, func=Act.Square)
                    junk = work.tile([PT, M], bf16, tag="junk")
                    nc.vector.scalar_tensor_tensor(
                        out=junk, in0=predc, scalar=thr_r[:, k:k + 1], in1=s2m,
                        op0=Alu.is_ge, op1=Alu.mult, accum_out=t1row[:, k:k + 1],
                    )
                    ind = work.tile([PT, M], bf16, tag="ind")
                    nc.vector.tensor_tensor(out=ind, in0=predc, in1=thrc, op=Alu.is_ge)
                    tcol = work.tile([PT, M], bf16, tag="tcol")
                    nc.vector.tensor_mul(tcol, ind, s2m)
                    for q in range(NQ):
                        nc.tensor.matmul(
                            t1c_ps[0:1, q * 512:(q + 1) * 512],
                            ones16,
                            tcol[:, q * 512:(q + 1) * 512],
                            start=(k == 0), stop=(k == CHR - 1),
                        )

                t1c_row = pb.tile([1, M], f32, tag="t1c_row")
                nc.scalar.copy(t1c_row, t1c_ps[0:1, :])
                nc.sync.dma_start(out=t1c_d[b:b + 1, :], in_=t1c_row)
                nc.sync.dma_start(
                    out=t1r_d[b].rearrange("(k p) -> p k", p=PT), in_=t1row
                )

    nc.compile()
    return nc


def _get_program():
    if "nc" not in _cache:
        _cache["nc"] = _build_program()
    return _cache["nc"]


def _gt_is_identity_perm(gt_perm, src_ns):
    """Exact check: gt_perm[b] == eye * (i < src_ns[b]), all entries in {0,1}."""
    if gt_perm.shape != (B, N, M):
        return False
    if gt_perm.min() < 0.0:
        return False
    i = np.arange(N)
    rowmask = (i[None, :] < src_ns[:, None]).astype(np.float32)  # [B, N]
    d = gt_perm[:, i, i]
    if not np.array_equal(d, rowmask):
        return False
    if not np.array_equal(gt_perm.sum(axis=2), rowmask):
        return False
    return True


def _reference_numpy(pred_dsmat, gt_perm, src_ns, tgt_ns, beta_value):
    """Direct numpy port of the reference — correctness fallback only."""
    out = 0.0
    n_sum = float(src_ns.astype(np.int64).sum())
    for b in range(pred_dsmat.shape[0]):
        p = pred_dsmat[b].astype(np.float64)
        g = gt_perm[b].astype(np.float64)
        s, t = int(src_ns[b]), int(tgt_ns[b])
        NN, MM = p.shape
        rm = (np.arange(NN) < s)
        cm = (np.arange(MM) < t)
        mask = rm[:, None] & cm[None, :]
        pred = np.clip(p, 0.0, 1.0) * mask
        gt = g * mask
        gp = pred * gt
        row_gt = gp.sum(1); col_gt = gp.sum(0)
        row_cnt = gt.sum(1); col_cnt = gt.sum(0)
        att_src = ((pred >= row_gt[:, None] - beta_value) & mask) * row_cnt[:, None]
        att_tgt = ((pred >= col_gt[None, :] - beta_value) & mask) * col_cnt[None, :]
        src_neg = (((att_src - gt) * pred) ** 2).sum(1)
        src_pos = (gp ** 2).sum(1)
        tgt_neg = (((att_tgt - gt) * pred) ** 2).sum(0)
        corr = (tgt_neg * col_cnt).sum()
        num = np.where(rm, src_pos, 1.0)
        den = np.where(rm, 1.0 + src_neg + corr, 1.0)
        out += -0.5 * (np.log(num / den) * rm).sum()
    return np.float32(out / n_sum)


def _host_prep(pred_dsmat, src_ns, tgt_ns, beta):
    ii = np.arange(N)
    rmask = (ii[None, :] < src_ns[:, None]).astype(np.float32)      # [B, N]
    diag = pred_dsmat[:, ii, ii].astype(np.float32)
    rowgt = np.clip(diag, 0.0, 1.0) * rmask                         # f32, exact
    srcpos = rowgt * rowgt
    thr = (rowgt - np.float32(beta)).astype(np.float32)             # [B, N]
    p16 = np.clip(pred_dsmat, 0.0, 1.0).astype(ml_dtypes.bfloat16)
    for gb in range(B):
        p16[gb, :, int(tgt_ns[gb]):] = 0                            # ragged col padding
        p16[gb, int(src_ns[gb]):, :] = 0                            # ragged row padding
    return rmask, srcpos, thr, p16


def _make_in_maps(p16, rmask, thr):
    thrc16 = thr.astype(ml_dtypes.bfloat16)
    in_maps = []
    for c in range(NCORES):
        b0 = c * BPC
        in_maps.append({
            "p16": np.ascontiguousarray(p16[b0:b0 + BPC]),
            "thr_r": np.ascontiguousarray(thr[b0:b0 + BPC, :NR]),
            "thrc16": np.ascontiguousarray(thrc16[b0:b0 + BPC]),
        })
    return in_maps


def _host_epilogue(t1row, t1col, srcpos, rmask, src_ns):
    """O(B*N) scalar epilogue on the device-computed threshold sums."""
    t1row = t1row.astype(np.float64)
    t1col = t1col.astype(np.float64)
    srcpos = srcpos.astype(np.float64)
    rmask = rmask.astype(np.float64)
    corr = ((t1col - srcpos) * rmask).sum(axis=1)                   # [B]
    src_neg = t1row - srcpos
    num = np.where(rmask > 0, np.maximum(srcpos, 1e-300), 1.0)
    den = np.where(rmask > 0, 1.0 + src_neg + corr[:, None], 1.0)
    total = -0.5 * (np.log(num / den) * rmask).sum()
    n_sum = float(src_ns.astype(np.int64).sum())
    return np.float32(total / n_sum)


def kernel(pred_dsmat, gt_perm, src_ns, tgt_ns, beta_value):
    pred_dsmat = np.asarray(pred_dsmat, dtype=np.float32)
    gt_perm = np.asarray(gt_perm, dtype=np.float32)
    src_ns = np.asarray(src_ns, dtype=np.int32)
    tgt_ns = np.asarray(tgt_ns, dtype=np.int32)
    beta = float(np.asarray(beta_value))

    if not _gt_is_identity_perm(gt_perm, src_ns) or int(src_ns.max()) > NR:
        return _reference_numpy(pred_dsmat, gt_perm, src_ns, tgt_ns, beta)

    from concourse.bass_utils import run_bass_kernel_spmd

    nc = _get_program()
    rmask, srcpos, thr, p16 = _host_prep(pred_dsmat, src_ns, tgt_ns, beta)
    in_maps = _make_in_maps(p16, rmask, thr)
    res = run_bass_kernel_spmd(nc, in_maps, list(range(NCORES)))
    t1row_c = np.concatenate([r["t1row"] for r in res.results], axis=0)  # [B, NR]
    t1row = np.zeros((B, N), np.float32)
    t1row[:, :NR] = t1row_c
    t1col = np.concatenate([r["t1col"] for r in res.results], axis=0)    # [B, M]
    return _host_epilogue(t1row, t1col, srcpos, rmask, src_ns)



# revision 7
# speedup vs baseline: 1.4240x; 1.4240x over previous
"""Trainium2 Bass kernel for nn_ContrastiveLossWithAttention.

Contract: kernel(**inputs) takes the FULL unsharded inputs (as produced by
reference.setup_inputs) and returns the FULL output (a float32 scalar).

Sharding: data parallel over the batch dim (B=16, 2 batches/core on 8
cores), with host-side batch re-pairing: batches are sorted by their valid
row count (ceil(src_ns/128) chunks) and each core gets one "big" + one
"small" batch, so the compiled per-core template (T1 chunks for slot 1,
T2 for slot 2) tracks the actual ragged work instead of the worst case.

Math (gt_perm verified host-side to be the identity permutation restricted
to rows i < src_ns; numpy fallback otherwise):
    thr[i]    = clip(p[i,i])-beta               (row & col thresholds equal)
    t1row[i]  = sum_j 1{pc >= thr[i]} * pc^2    (pc = clip(pred)*valid mask)
    t1col[j]  = sum_i 1{pc >= thr[j]} * pc^2    (only needed for j < src_ns)
    loss_b    = -0.5 sum_i [ln(rowgt_i^2) - ln(1 + (t1row_i - rowgt_i^2)
                 + sum_{j<s}(t1col_j - rowgt_j^2))]

Device mapping per 128-row chunk (engine-balanced):
    DVE : c01 = tensor_scalar(p >= thr_row)          (fast ~4x mode)
          u   = c01 * p                              (tensor_tensor, 2x)
          w   = custom fused (p >= thrc)*p^2         (registered DVE ucode op,
                cols [0, MC_slot) only -- t1col is unused for j >= src_ns)
    ACT : Square(u) + row-accumulate -> t1row        (frees DVE of the 1x STT)
    PE  : ones^T @ w column sums accumulated in PSUM across chunks
p16 is stored host-side in chunk-major layout so each piece-DMA reads long
contiguous HBM lines (8KB+ packets) instead of 4KB partition rows.
"""

import numpy as np
import ml_dtypes

B, N, M = 16, 2048, 2048
NCORES = 8
PT = 128               # partitions
MAXCH = 12             # src_ns < 1537 always => rows beyond 1536 never valid
NR = PT * MAXCH
PIECE = 2              # chunks per piece-DMA

_cache = {}
_COLOP = {}


def _get_colop():
    """Register (once) the fused column op: out = ((Src0 >= Src1) * Src0)^2.

    Computes 1{p >= thrc} * p^2 in ONE 1x DVE pass (vs. an is_ge + mult
    tensor_tensor pair), with no dependency on a separate Square tensor.
    """
    if "op" in _COLOP:
        return _COLOP["op"]
    from concourse import dve_ops
    from concourse.dve_spec import Spec, Src0, Src1, sq, lower
    from concourse.dve_spec import _has_src1 as has_src1
    from concourse.dve_uop import DveOpSpec

    name = "TSQ_GE_COL_ANT"

    def _ref(in0, in1, s0, s1, imm2):
        a = in0.astype(np.float32)
        b = in1.astype(np.float32)
        return ((a >= b) * a) ** 2

    spec = Spec(body=sq((Src0 >= Src1) * Src0), reference=_ref)
    existing = {op.name for op in dve_ops.OPS}
    if name not in existing:
        row = max(dve_ops._SUB_OPCODE_FOR_NAME.values()) + 1
        assert row < 0x20
        dve_ops._SUB_OPCODE_FOR_NAME[name] = row
        shas = {}
        for ver in ("v3", "v4"):
            tmp = DveOpSpec(
                name=name, opcode=row, uops=lower(spec, ver=ver),
                rd1_en=has_src1(spec),
            )
            shas[ver] = tmp.sha(ver)
        op = dve_ops.DveOp(name, spec, subdim=False, uops_sha=shas)
        dve_ops.OPS.append(op)
        dve_ops.CUSTOM_DVE_SPECS[name] = spec
    else:
        op = next(o for o in dve_ops.OPS if o.name == name)
    _COLOP["op"] = op
    return op


def _build_program(T1, T2, MC1, MC2):
    import concourse.tile as tile
    from concourse import bacc, mybir

    f32 = mybir.dt.float32
    bf16 = mybir.dt.bfloat16
    Alu = mybir.AluOpType
    Act = mybir.ActivationFunctionType
    colop = _get_colop()

    TT_ = T1 + T2
    NPC = (TT_ + PIECE - 1) // PIECE

    nc = bacc.Bacc("TRN2", debug=False, num_devices=NCORES)

    p_d = nc.dram_tensor("p16", [PT, TT_ * M], bf16, kind="ExternalInput")
    thr_d = nc.dram_tensor("thr_r", [PT, TT_], f32, kind="ExternalInput")
    thc_d = nc.dram_tensor("thrc16", [2, MC1], bf16, kind="ExternalInput")
    t1r_d = nc.dram_tensor("t1row", [PT, TT_], f32, kind="ExternalOutput")
    t1c_d = nc.dram_tensor("t1col", [2, MC1], f32, kind="ExternalOutput")

    with tile.TileContext(nc) as tc:
        with (
            tc.tile_pool(name="consts", bufs=1) as consts,
            tc.tile_pool(name="pieces", bufs=6) as pieces,
            tc.tile_pool(name="work", bufs=3) as work,
            tc.tile_pool(name="ps_col", bufs=1, space="PSUM") as ps_col,
        ):
            ones16 = consts.tile([PT, 1], bf16, tag="ones16")
            nc.vector.memset(ones16, 1.0)
            thr_r = consts.tile([PT, TT_], f32, tag="thr_r")
            nc.sync.dma_start(out=thr_r, in_=thr_d[:, :])
            tcb1 = consts.tile([PT, MC1], bf16, tag="tcb1")
            nc.sync.dma_start(out=tcb1, in_=thc_d[0:1, :].to_broadcast([PT, MC1]))
            tcb2 = consts.tile([PT, MC2], bf16, tag="tcb2")
            nc.sync.dma_start(out=tcb2, in_=thc_d[1:2, :MC2].to_broadcast([PT, MC2]))
            t1row = consts.tile([PT, TT_], f32, tag="t1row")

            ptiles = []
            for pc in range(NPC):
                c0, c1 = pc * PIECE, min((pc + 1) * PIECE, TT_)
                pt_ = pieces.tile([PT, (c1 - c0) * M], bf16, tag="piece")
                nc.sync.dma_start(out=pt_, in_=p_d[:, c0 * M:c1 * M])
                ptiles.append(pt_)

            for s, (Ts, MCs, tcb, k0) in enumerate(
                [(T1, MC1, tcb1, 0), (T2, MC2, tcb2, T1)]
            ):
                t1c_ps = ps_col.tile([1, MCs], f32, tag=f"t1c{s}")
                for kk in range(Ts):
                    k = k0 + kk
                    pk = ptiles[k // PIECE][:, (k % PIECE) * M:(k % PIECE + 1) * M]
                    c01 = work.tile([PT, M], bf16, tag="c01")
                    nc.vector.tensor_scalar(
                        c01, pk, thr_r[:, k:k + 1], None, op0=Alu.is_ge
                    )
                    u = work.tile([PT, M], bf16, tag="u")
                    nc.vector.tensor_mul(u, c01, pk)
                    usq = work.tile([PT, M], bf16, tag="usq")
                    nc.scalar.activation(
                        out=usq, in_=u, func=Act.Square,
                        accum_out=t1row[:, k:k + 1],
                    )
                    w = work.tile([PT, MCs], bf16, tag="w")
                    nc.vector._custom_dve(
                        colop, out=w, in0=pk[:, :MCs], in1=tcb[:, :MCs]
                    )
                    for q0 in range(0, MCs, 512):
                        q1 = min(q0 + 512, MCs)
                        nc.tensor.matmul(
                            t1c_ps[0:1, q0:q1], ones16, w[:, q0:q1],
                            start=(kk == 0), stop=(kk == Ts - 1),
                        )
                t1c_row = work.tile([1, MCs], f32, tag=f"t1c_row{s}")
                nc.scalar.copy(t1c_row, t1c_ps[0:1, :])
                nc.sync.dma_start(out=t1c_d[s:s + 1, :MCs], in_=t1c_row)

            nc.sync.dma_start(out=t1r_d[:, :], in_=t1row)

    nc.compile()
    return nc


def _get_program(T1, T2, MC1, MC2):
    key = (T1, T2, MC1, MC2)
    if key not in _cache:
        _cache[key] = _build_program(*key)
    return _cache[key]


def _template(src_ns):
    """Host-side batch re-pairing: 8 'big' + 8 'small' batches by chunk
    count; per-slot chunk counts and column widths from the actual data."""
    ch = np.maximum(1, np.ceil(src_ns / PT).astype(int))
    order = np.argsort(-ch, kind="stable")
    bigs, smalls = order[:NCORES], order[NCORES:]
    T1 = int(ch[bigs].max())
    T2 = int(ch[smalls].max())
    MC1 = int(np.ceil(src_ns[bigs].max() / 64) * 64)
    MC2 = int(np.ceil(src_ns[smalls].max() / 64) * 64)
    return T1, T2, MC1, MC2, bigs, smalls


def _gt_is_identity_perm(gt_perm, src_ns):
    """Exact check: gt_perm[b] == eye * (i < src_ns[b]), all entries in {0,1}."""
    if gt_perm.shape != (B, N, M):
        return False
    if gt_perm.min() < 0.0:
        return False
    i = np.arange(N)
    rowmask = (i[None, :] < src_ns[:, None]).astype(np.float32)  # [B, N]
    d = gt_perm[:, i, i]
    if not np.array_equal(d, rowmask):
        return False
    if not np.array_equal(gt_perm.sum(axis=2), rowmask):
        return False
    return True


def _reference_numpy(pred_dsmat, gt_perm, src_ns, tgt_ns, beta_value):
    """Direct numpy port of the reference — correctness fallback only."""
    out = 0.0
    n_sum = float(src_ns.astype(np.int64).sum())
    for b in range(pred_dsmat.shape[0]):
        p = pred_dsmat[b].astype(np.float64)
        g = gt_perm[b].astype(np.float64)
        s, t = int(src_ns[b]), int(tgt_ns[b])
        NN, MM = p.shape
        rm = (np.arange(NN) < s)
        cm = (np.arange(MM) < t)
        mask = rm[:, None] & cm[None, :]
        pred = np.clip(p, 0.0, 1.0) * mask
        gt = g * mask
        gp = pred * gt
        row_gt = gp.sum(1); col_gt = gp.sum(0)
        row_cnt = gt.sum(1); col_cnt = gt.sum(0)
        att_src = ((pred >= row_gt[:, None] - beta_value) & mask) * row_cnt[:, None]
        att_tgt = ((pred >= col_gt[None, :] - beta_value) & mask) * col_cnt[None, :]
        src_neg = (((att_src - gt) * pred) ** 2).sum(1)
        src_pos = (gp ** 2).sum(1)
        tgt_neg = (((att_tgt - gt) * pred) ** 2).sum(0)
        corr = (tgt_neg * col_cnt).sum()
        num = np.where(rm, src_pos, 1.0)
        den = np.where(rm, 1.0 + src_neg + corr, 1.0)
        out += -0.5 * (np.log(num / den) * rm).sum()
    return np.float32(out / n_sum)


def _host_prep(pred_dsmat, src_ns, tgt_ns, beta):
    ii = np.arange(N)
    rmask = (ii[None, :] < src_ns[:, None]).astype(np.float32)      # [B, N]
    diag = pred_dsmat[:, ii, ii].astype(np.float32)
    rowgt = np.clip(diag, 0.0, 1.0) * rmask                         # f32, exact
    srcpos = rowgt * rowgt
    thr = (rowgt - np.float32(beta)).astype(np.float32)             # [B, N]
    p16 = np.clip(pred_dsmat, 0.0, 1.0).astype(ml_dtypes.bfloat16)
    for gb in range(B):
        p16[gb, :, int(tgt_ns[gb]):] = 0                            # ragged col padding
        p16[gb, int(src_ns[gb]):, :] = 0                            # ragged row padding
    return rmask, srcpos, thr, p16


def _make_in_maps(p16, thr, src_ns):
    T1, T2, MC1, MC2, bigs, smalls = _template(src_ns)
    TT_ = T1 + T2
    thr16 = thr.astype(ml_dtypes.bfloat16)
    in_maps = []
    for c in range(NCORES):
        chunks = [(int(bigs[c]), kk) for kk in range(T1)] + \
                 [(int(smalls[c]), kk) for kk in range(T2)]
        parr = np.empty((PT, TT_ * M), dtype=ml_dtypes.bfloat16)
        tharr = np.empty((PT, TT_), dtype=np.float32)
        for kk, (gb, ck) in enumerate(chunks):
            parr[:, kk * M:(kk + 1) * M] = p16[gb, ck * PT:(ck + 1) * PT, :]
            tharr[:, kk] = thr[gb, ck * PT:(ck + 1) * PT]
        thc = np.zeros((2, MC1), dtype=ml_dtypes.bfloat16)
        thc[0, :MC1] = thr16[bigs[c], :MC1]
        thc[1, :MC2] = thr16[smalls[c], :MC2]
        in_maps.append({"p16": parr, "thr_r": tharr, "thrc16": thc})
    return (T1, T2, MC1, MC2, bigs, smalls), in_maps


def _host_epilogue(t1row, t1col, srcpos, rmask, src_ns):
    """O(B*N) scalar epilogue on the device-computed threshold sums."""
    t1row = t1row.astype(np.float64)
    t1col = t1col.astype(np.float64)
    srcpos = srcpos.astype(np.float64)
    rmask = rmask.astype(np.float64)
    corr = ((t1col - srcpos) * rmask).sum(axis=1)                   # [B]
    src_neg = t1row - srcpos
    num = np.where(rmask > 0, np.maximum(srcpos, 1e-300), 1.0)
    den = np.where(rmask > 0, 1.0 + src_neg + corr[:, None], 1.0)
    total = -0.5 * (np.log(num / den) * rmask).sum()
    n_sum = float(src_ns.astype(np.int64).sum())
    return np.float32(total / n_sum)


def _gather_outputs(res, tpl):
    T1, T2, MC1, MC2, bigs, smalls = tpl
    t1row = np.zeros((B, N), np.float32)
    t1col = np.zeros((B, M), np.float32)
    for c in range(NCORES):
        r = res.results[c]
        tr = r["t1row"]                                             # [PT, T1+T2]
        for kk in range(T1):
            t1row[bigs[c], kk * PT:(kk + 1) * PT] = tr[:, kk]
        for kk in range(T2):
            t1row[smalls[c], kk * PT:(kk + 1) * PT] = tr[:, T1 + kk]
        t1col[bigs[c], :MC1] = r["t1col"][0, :MC1]
        t1col[smalls[c], :MC2] = r["t1col"][1, :MC2]
    return t1row, t1col


def kernel(pred_dsmat, gt_perm, src_ns, tgt_ns, beta_value):
    pred_dsmat = np.asarray(pred_dsmat, dtype=np.float32)
    gt_perm = np.asarray(gt_perm, dtype=np.float32)
    src_ns = np.asarray(src_ns, dtype=np.int32)
    tgt_ns = np.asarray(tgt_ns, dtype=np.int32)
    beta = float(np.asarray(beta_value))

    if not _gt_is_identity_perm(gt_perm, src_ns) or int(src_ns.max()) > NR:
        return _reference_numpy(pred_dsmat, gt_perm, src_ns, tgt_ns, beta)

    from concourse.bass_utils import run_bass_kernel_spmd

    rmask, srcpos, thr, p16 = _host_prep(pred_dsmat, src_ns, tgt_ns, beta)
    tpl, in_maps = _make_in_maps(p16, thr, src_ns)
    nc = _get_program(*tpl[:4])
    res = run_bass_kernel_spmd(nc, in_maps, list(range(NCORES)))
    t1row, t1col = _gather_outputs(res, tpl)
    return _host_epilogue(t1row, t1col, srcpos, rmask, src_ns)


# revision 10
# speedup vs baseline: 1.4708x; 1.0329x over previous
"""Trainium2 Bass kernel for nn_ContrastiveLossWithAttention.

Contract: kernel(**inputs) takes the FULL unsharded inputs (as produced by
reference.setup_inputs) and returns the FULL output (a float32 scalar).

Sharding: data parallel over the batch dim (B=16, 2 batches/core on 8
cores), with host-side batch re-pairing: batches are sorted by their valid
row count (ceil(src_ns/128) chunks) and each core gets one "big" + one
"small" batch, so the compiled per-core template (T1 chunks for slot 1,
T2 for slot 2) tracks the actual ragged work instead of the worst case.

Math (gt_perm verified host-side to be the identity permutation restricted
to rows i < src_ns; numpy fallback otherwise):
    thr[i]    = clip(p[i,i])-beta               (row & col thresholds equal)
    t1row[i]  = sum_j 1{pc >= thr[i]} * pc^2    (pc = clip(pred)*valid mask)
    t1col[j]  = sum_i 1{pc >= thr[j]} * pc^2    (only needed for j < src_ns)
    loss_b    = -0.5 sum_i [ln(rowgt_i^2) - ln(1 + (t1row_i - rowgt_i^2)
                 + sum_{j<s}(t1col_j - rowgt_j^2))]

Device mapping per 128-row chunk (engine-balanced):
    DVE : c01 = tensor_scalar(p >= thr_row)          (fast ~4x mode)
          u   = c01 * p                              (tensor_tensor, 2x)
          w   = custom fused (p >= thrc)*p^2         (registered DVE ucode op,
                cols [0, MC_slot) only -- t1col is unused for j >= src_ns)
    ACT : Square(u) + row-accumulate -> t1row        (frees DVE of the 1x STT)
    PE  : ones^T @ w column sums accumulated in PSUM across chunks
p16 is stored host-side in chunk-major layout so each piece-DMA reads long
contiguous HBM lines (8KB+ packets) instead of 4KB partition rows.
"""

import numpy as np
import ml_dtypes

B, N, M = 16, 2048, 2048
NCORES = 8
PT = 128               # partitions
MAXCH = 12             # src_ns < 1537 always => rows beyond 1536 never valid
NR = PT * MAXCH
PIECE = 2              # chunks per piece-DMA

_cache = {}
_COLOP = {}


def _get_colop():
    """Register (once) the fused column op: out = ((Src0 >= Src1) * Src0)^2.

    Computes 1{p >= thrc} * p^2 in ONE 1x DVE pass (vs. an is_ge + mult
    tensor_tensor pair), with no dependency on a separate Square tensor.
    """
    if "op" in _COLOP:
        return _COLOP["op"]
    from concourse import dve_ops
    from concourse.dve_spec import Spec, Src0, Src1, sq, lower
    from concourse.dve_spec import _has_src1 as has_src1
    from concourse.dve_uop import DveOpSpec

    name = "TSQ_GE_COL_ANT"

    def _ref(in0, in1, s0, s1, imm2):
        a = in0.astype(np.float32)
        b = in1.astype(np.float32)
        return ((a >= b) * a) ** 2

    spec = Spec(body=sq((Src0 >= Src1) * Src0), reference=_ref)
    existing = {op.name for op in dve_ops.OPS}
    if name not in existing:
        row = max(dve_ops._SUB_OPCODE_FOR_NAME.values()) + 1
        assert row < 0x20
        dve_ops._SUB_OPCODE_FOR_NAME[name] = row
        shas = {}
        for ver in ("v3", "v4"):
            tmp = DveOpSpec(
                name=name, opcode=row, uops=lower(spec, ver=ver),
                rd1_en=has_src1(spec),
            )
            shas[ver] = tmp.sha(ver)
        op = dve_ops.DveOp(name, spec, subdim=False, uops_sha=shas)
        dve_ops.OPS.append(op)
        dve_ops.CUSTOM_DVE_SPECS[name] = spec
    else:
        op = next(o for o in dve_ops.OPS if o.name == name)
    _COLOP["op"] = op
    return op


def _build_program(T1, T2, MC1, MC2):
    import concourse.tile as tile
    from concourse import bacc, mybir

    f32 = mybir.dt.float32
    bf16 = mybir.dt.bfloat16
    Alu = mybir.AluOpType
    Act = mybir.ActivationFunctionType
    colop = _get_colop()

    TT_ = T1 + T2
    NPC = (TT_ + PIECE - 1) // PIECE

    nc = bacc.Bacc("TRN2", debug=False, num_devices=NCORES)

    p_d = nc.dram_tensor("p16", [PT, TT_ * M], bf16, kind="ExternalInput")
    thr_d = nc.dram_tensor("thr_r", [PT, TT_], f32, kind="ExternalInput")
    thc_d = nc.dram_tensor("thrc16", [2, MC1], bf16, kind="ExternalInput")
    t1r_d = nc.dram_tensor("t1row", [PT, TT_], f32, kind="ExternalOutput")
    t1c_d = nc.dram_tensor("t1col", [2, MC1], f32, kind="ExternalOutput")

    with tile.TileContext(nc) as tc:
        with (
            tc.tile_pool(name="consts", bufs=1) as consts,
            tc.tile_pool(name="pieces", bufs=7) as pieces,
            tc.tile_pool(name="work", bufs=4) as work,
            tc.tile_pool(name="ps_col", bufs=1, space="PSUM") as ps_col,
        ):
            # piece 0 is a single chunk issued first so compute starts ASAP;
            # the tcb broadcasts ride the ACT engine's separate HWDGE ring to
            # run in parallel with the piece loads on the SP ring.
            bounds = [0, 1]
            while bounds[-1] < TT_:
                bounds.append(min(bounds[-1] + PIECE, TT_))
            ptile_of, poff_of, ptiles = {}, {}, []
            pt0 = pieces.tile([PT, M], bf16, tag="piece")
            nc.sync.dma_start(out=pt0, in_=p_d[:, 0:M])
            ptiles.append(pt0)
            for k in range(bounds[0], bounds[1]):
                ptile_of[k], poff_of[k] = pt0, k - bounds[0]

            ones16 = consts.tile([PT, 1], bf16, tag="ones16")
            nc.vector.memset(ones16, 1.0)
            thr_r = consts.tile([PT, TT_], f32, tag="thr_r")
            nc.sync.dma_start(out=thr_r, in_=thr_d[:, :])
            tcb1 = consts.tile([PT, MC1], bf16, tag="tcb1")
            nc.scalar.dma_start(out=tcb1, in_=thc_d[0:1, :].to_broadcast([PT, MC1]))
            tcb2 = consts.tile([PT, MC2], bf16, tag="tcb2")
            nc.scalar.dma_start(out=tcb2, in_=thc_d[1:2, :MC2].to_broadcast([PT, MC2]))
            t1row = consts.tile([PT, TT_], f32, tag="t1row")

            for pc in range(1, len(bounds) - 1):
                c0, c1 = bounds[pc], bounds[pc + 1]
                pt_ = pieces.tile([PT, (c1 - c0) * M], bf16, tag="piece")
                nc.sync.dma_start(out=pt_, in_=p_d[:, c0 * M:c1 * M])
                ptiles.append(pt_)
                for k in range(c0, c1):
                    ptile_of[k], poff_of[k] = pt_, k - c0

            for s, (Ts, MCs, tcb, k0) in enumerate(
                [(T1, MC1, tcb1, 0), (T2, MC2, tcb2, T1)]
            ):
                t1c_ps = ps_col.tile([1, MCs], f32, tag=f"t1c{s}")
                for kk in range(Ts):
                    k = k0 + kk
                    pk = ptile_of[k][:, poff_of[k] * M:(poff_of[k] + 1) * M]
                    w = work.tile([PT, MCs], bf16, tag="w")
                    nc.vector._custom_dve(
                        colop, out=w, in0=pk[:, :MCs], in1=tcb[:, :MCs]
                    )
                    c01 = work.tile([PT, M], bf16, tag="c01")
                    nc.vector.tensor_scalar(
                        c01, pk, thr_r[:, k:k + 1], None, op0=Alu.is_ge
                    )
                    u = work.tile([PT, M], bf16, tag="u")
                    nc.vector.tensor_mul(u, c01, pk)
                    usq = work.tile([PT, M], bf16, tag="usq")
                    nc.scalar.activation(
                        out=usq, in_=u, func=Act.Square,
                        accum_out=t1row[:, k:k + 1],
                    )
                    for q0 in range(0, MCs, 512):
                        q1 = min(q0 + 512, MCs)
                        nc.tensor.matmul(
                            t1c_ps[0:1, q0:q1], ones16, w[:, q0:q1],
                            start=(kk == 0), stop=(kk == Ts - 1),
                        )
                t1c_row = work.tile([1, MCs], f32, tag=f"t1c_row{s}")
                nc.scalar.copy(t1c_row, t1c_ps[0:1, :])
                nc.sync.dma_start(out=t1c_d[s:s + 1, :MCs], in_=t1c_row)

            nc.sync.dma_start(out=t1r_d[:, :], in_=t1row)

    nc.compile()
    return nc


def _get_program(T1, T2, MC1, MC2):
    key = (T1, T2, MC1, MC2)
    if key not in _cache:
        _cache[key] = _build_program(*key)
    return _cache[key]


def _template(src_ns):
    """Host-side batch re-pairing: 8 'big' + 8 'small' batches by chunk
    count; per-slot chunk counts and column widths from the actual data."""
    ch = np.maximum(1, np.ceil(src_ns / PT).astype(int))
    order = np.argsort(-ch, kind="stable")
    bigs, smalls = order[:NCORES], order[NCORES:]
    T1 = int(ch[bigs].max())
    T2 = int(ch[smalls].max())
    MC1 = int(np.ceil(src_ns[bigs].max() / 64) * 64)
    MC2 = int(np.ceil(src_ns[smalls].max() / 64) * 64)
    return T1, T2, MC1, MC2, bigs, smalls


def _gt_is_identity_perm(gt_perm, src_ns):
    """Exact check: gt_perm[b] == eye * (i < src_ns[b]), all entries in {0,1}."""
    if gt_perm.shape != (B, N, M):
        return False
    if gt_perm.min() < 0.0:
        return False
    i = np.arange(N)
    rowmask = (i[None, :] < src_ns[:, None]).astype(np.float32)  # [B, N]
    d = gt_perm[:, i, i]
    if not np.array_equal(d, rowmask):
        return False
    if not np.array_equal(gt_perm.sum(axis=2), rowmask):
        return False
    return True


def _reference_numpy(pred_dsmat, gt_perm, src_ns, tgt_ns, beta_value):
    """Direct numpy port of the reference — correctness fallback only."""
    out = 0.0
    n_sum = float(src_ns.astype(np.int64).sum())
    for b in range(pred_dsmat.shape[0]):
        p = pred_dsmat[b].astype(np.float64)
        g = gt_perm[b].astype(np.float64)
        s, t = int(src_ns[b]), int(tgt_ns[b])
        NN, MM = p.shape
        rm = (np.arange(NN) < s)
        cm = (np.arange(MM) < t)
        mask = rm[:, None] & cm[None, :]
        pred = np.clip(p, 0.0, 1.0) * mask
        gt = g * mask
        gp = pred * gt
        row_gt = gp.sum(1); col_gt = gp.sum(0)
        row_cnt = gt.sum(1); col_cnt = gt.sum(0)
        att_src = ((pred >= row_gt[:, None] - beta_value) & mask) * row_cnt[:, None]
        att_tgt = ((pred >= col_gt[None, :] - beta_value) & mask) * col_cnt[None, :]
        src_neg = (((att_src - gt) * pred) ** 2).sum(1)
        src_pos = (gp ** 2).sum(1)
        tgt_neg = (((att_tgt - gt) * pred) ** 2).sum(0)
        corr = (tgt_neg * col_cnt).sum()
        num = np.where(rm, src_pos, 1.0)
        den = np.where(rm, 1.0 + src_neg + corr, 1.0)
        out += -0.5 * (np.log(num / den) * rm).sum()
    return np.float32(out / n_sum)


def _host_prep(pred_dsmat, src_ns, tgt_ns, beta):
    ii = np.arange(N)
    rmask = (ii[None, :] < src_ns[:, None]).astype(np.float32)      # [B, N]
    diag = pred_dsmat[:, ii, ii].astype(np.float32)
    rowgt = np.clip(diag, 0.0, 1.0) * rmask                         # f32, exact
    srcpos = rowgt * rowgt
    thr = (rowgt - np.float32(beta)).astype(np.float32)             # [B, N]
    p16 = np.clip(pred_dsmat, 0.0, 1.0).astype(ml_dtypes.bfloat16)
    for gb in range(B):
        p16[gb, :, int(tgt_ns[gb]):] = 0                            # ragged col padding
        p16[gb, int(src_ns[gb]):, :] = 0                            # ragged row padding
    return rmask, srcpos, thr, p16


def _make_in_maps(p16, thr, src_ns):
    T1, T2, MC1, MC2, bigs, smalls = _template(src_ns)
    TT_ = T1 + T2
    thr16 = thr.astype(ml_dtypes.bfloat16)
    in_maps = []
    for c in range(NCORES):
        chunks = [(int(bigs[c]), kk) for kk in range(T1)] + \
                 [(int(smalls[c]), kk) for kk in range(T2)]
        parr = np.empty((PT, TT_ * M), dtype=ml_dtypes.bfloat16)
        tharr = np.empty((PT, TT_), dtype=np.float32)
        for kk, (gb, ck) in enumerate(chunks):
            parr[:, kk * M:(kk + 1) * M] = p16[gb, ck * PT:(ck + 1) * PT, :]
            tharr[:, kk] = thr[gb, ck * PT:(ck + 1) * PT]
        thc = np.zeros((2, MC1), dtype=ml_dtypes.bfloat16)
        thc[0, :MC1] = thr16[bigs[c], :MC1]
        thc[1, :MC2] = thr16[smalls[c], :MC2]
        in_maps.append({"p16": parr, "thr_r": tharr, "thrc16": thc})
    return (T1, T2, MC1, MC2, bigs, smalls), in_maps


def _host_epilogue(t1row, t1col, srcpos, rmask, src_ns):
    """O(B*N) scalar epilogue on the device-computed threshold sums."""
    t1row = t1row.astype(np.float64)
    t1col = t1col.astype(np.float64)
    srcpos = srcpos.astype(np.float64)
    rmask = rmask.astype(np.float64)
    corr = ((t1col - srcpos) * rmask).sum(axis=1)                   # [B]
    src_neg = t1row - srcpos
    num = np.where(rmask > 0, np.maximum(srcpos, 1e-300), 1.0)
    den = np.where(rmask > 0, 1.0 + src_neg + corr[:, None], 1.0)
    total = -0.5 * (np.log(num / den) * rmask).sum()
    n_sum = float(src_ns.astype(np.int64).sum())
    return np.float32(total / n_sum)


def _gather_outputs(res, tpl):
    T1, T2, MC1, MC2, bigs, smalls = tpl
    t1row = np.zeros((B, N), np.float32)
    t1col = np.zeros((B, M), np.float32)
    for c in range(NCORES):
        r = res.results[c]
        tr = r["t1row"]                                             # [PT, T1+T2]
        for kk in range(T1):
            t1row[bigs[c], kk * PT:(kk + 1) * PT] = tr[:, kk]
        for kk in range(T2):
            t1row[smalls[c], kk * PT:(kk + 1) * PT] = tr[:, T1 + kk]
        t1col[bigs[c], :MC1] = r["t1col"][0, :MC1]
        t1col[smalls[c], :MC2] = r["t1col"][1, :MC2]
    return t1row, t1col


def kernel(pred_dsmat, gt_perm, src_ns, tgt_ns, beta_value):
    pred_dsmat = np.asarray(pred_dsmat, dtype=np.float32)
    gt_perm = np.asarray(gt_perm, dtype=np.float32)
    src_ns = np.asarray(src_ns, dtype=np.int32)
    tgt_ns = np.asarray(tgt_ns, dtype=np.int32)
    beta = float(np.asarray(beta_value))

    if not _gt_is_identity_perm(gt_perm, src_ns) or int(src_ns.max()) > NR:
        return _reference_numpy(pred_dsmat, gt_perm, src_ns, tgt_ns, beta)

    from concourse.bass_utils import run_bass_kernel_spmd

    rmask, srcpos, thr, p16 = _host_prep(pred_dsmat, src_ns, tgt_ns, beta)
    tpl, in_maps = _make_in_maps(p16, thr, src_ns)
    nc = _get_program(*tpl[:4])
    res = run_bass_kernel_spmd(nc, in_maps, list(range(NCORES)))
    t1row, t1col = _gather_outputs(res, tpl)
    return _host_epilogue(t1row, t1col, srcpos, rmask, src_ns)
